# revision 2
# baseline (speedup 1.0000x reference)
"""Bass/Tile kernel for the DGE transformer block (per-core shard).

Per core: 4 batch elements => rows M = 4*256 = 1024, D=1024, H=16, Dh=64, DFF=4096.

Layouts: "natural" [rows on partitions, features free]; "T" [features on
partitions, rows free].  matmul: out[M,N] = lhsT[K,M].T @ rhs[K,N].

v3: proj interleaved into the attention loop per batch (keeps PE fed during
softmax chains), weight-pool allocs hoisted for DMA prefetch (nested LIFO),
X1 loaded at kernel start, psum bufs retuned, qk bias-add moved to DVE.
"""

import math

import sys as _sys
for _p in ("/opt/trn_rl_repo", "/root/.axon_site/_ro/trn_rl_repo"):
    if _p not in _sys.path:
        _sys.path.insert(0, _p)

import concourse.bass as bass
import concourse.mybir as mybir
import concourse.tile as tile
from concourse.masks import make_identity

F32 = mybir.dt.float32
BF16 = mybir.dt.bfloat16
AF = mybir.ActivationFunctionType
ALU = mybir.AluOpType

D = 1024
H = 16
DH = 64
DFF = 4096
NB = 4          # batches per core
C = 256         # context length
M = NB * C      # rows per core = 1024
P = 128
RT = M // P     # 8 row tiles
DC = D // P     # 8 d-chunks
FC = DFF // P   # 32 dff chunks
EPS = 1e-5
SCALE = 1.0 / math.sqrt(DH)


def build_kernel(nc, repeat=1):
    # ---- DRAM I/O ----
    X = nc.dram_tensor("X", [M, D], F32, kind="ExternalInput").ap()
    W_qkv = nc.dram_tensor("W_qkv", [D, 3 * D], BF16, kind="ExternalInput").ap()
    b_qkv = nc.dram_tensor("b_qkv", [3 * D], F32, kind="ExternalInput").ap()
    W_proj = nc.dram_tensor("W_proj", [D, D], BF16, kind="ExternalInput").ap()
    b_proj = nc.dram_tensor("b_proj", [D], F32, kind="ExternalInput").ap()
    g1 = nc.dram_tensor("g1", [D], F32, kind="ExternalInput").ap()
    beta1 = nc.dram_tensor("beta1", [D], F32, kind="ExternalInput").ap()
    g2 = nc.dram_tensor("g2", [D], F32, kind="ExternalInput").ap()
    beta2 = nc.dram_tensor("beta2", [D], F32, kind="ExternalInput").ap()
    W_ff1 = nc.dram_tensor("W_ff1", [D, DFF], BF16, kind="ExternalInput").ap()
    b_ff1 = nc.dram_tensor("b_ff1", [DFF], F32, kind="ExternalInput").ap()
    W_ff2 = nc.dram_tensor("W_ff2", [DFF, D], BF16, kind="ExternalInput").ap()
    b_ff2 = nc.dram_tensor("b_ff2", [D], F32, kind="ExternalInput").ap()

    X_out = nc.dram_tensor("X_out", [M, D], F32, kind="ExternalOutput").ap()
    A_out = nc.dram_tensor("A_out", [M, C], F32, kind="ExternalOutput").ap()

    Wqkv_r = W_qkv.rearrange("(kc p) n -> p kc n", p=P)    # [128, DC, 3072]
    Wproj_r = W_proj.rearrange("(kc p) n -> p kc n", p=P)  # [128, DC, 1024]
    Wff1_r = W_ff1.rearrange("(kc p) n -> p kc n", p=P)    # [128, DC, 4096]
    Wff2_r = W_ff2.rearrange("(kc p) n -> p kc n", p=P)    # [128, FC, 1024]
    X_r = X.rearrange("(rt p) n -> p rt n", p=P)           # [128, RT, 1024]

    with tile.TileContext(nc) as tc:
      for _rep in range(repeat):
        # ---------------- whole-kernel pools ----------------
        p_ident = tc.alloc_tile_pool(name="p_ident", bufs=1)
        ident = p_ident.tile([P, P], BF16)
        make_identity(nc, ident)
        # partition-major per-feature vectors: [128, n_chunks] each
        gb1_pm = p_ident.tile([P, DC], F32)
        nc.sync.dma_start(out=gb1_pm, in_=g1.rearrange("(c p) -> p c", p=P))
        bb1_pm = p_ident.tile([P, DC], F32)
        nc.sync.dma_start(out=bb1_pm, in_=beta1.rearrange("(c p) -> p c", p=P))
        gb2_pm = p_ident.tile([P, DC], F32)
        nc.sync.dma_start(out=gb2_pm, in_=g2.rearrange("(c p) -> p c", p=P))
        bb2_pm = p_ident.tile([P, DC], F32)
        nc.sync.dma_start(out=bb2_pm, in_=beta2.rearrange("(c p) -> p c", p=P))
        bqk_pm = p_ident.tile([P, 16], F32)
        nc.sync.dma_start(out=bqk_pm,
                          in_=b_qkv[0:2 * D].rearrange("(c p) -> p c", p=P))
        bff1_pm = p_ident.tile([P, FC], F32)
        nc.sync.dma_start(out=bff1_pm, in_=b_ff1.rearrange("(c p) -> p c", p=P))
        # broadcast bias tiles, built on-chip from single-row DMAs
        bv_bc = p_ident.tile([P, D], F32)
        nc.sync.dma_start(out=bv_bc[0:1, :], in_=b_qkv[2 * D:3 * D][None, :])
        nc.gpsimd.partition_broadcast(bv_bc, bv_bc[0:1, :])
        bproj_bc = p_ident.tile([P, D], F32)
        nc.sync.dma_start(out=bproj_bc[0:1, :], in_=b_proj[None, :])
        nc.gpsimd.partition_broadcast(bproj_bc, bproj_bc[0:1, :])
        bff2_bc = p_ident.tile([P, D], F32)
        nc.sync.dma_start(out=bff2_bc[0:1, :], in_=b_ff2[None, :])
        nc.gpsimd.partition_broadcast(bff2_bc, bff2_bc[0:1, :])

        psum = tc.alloc_tile_pool(name="psum", bufs=2, space="PSUM")
        small = tc.alloc_tile_pool(name="small", bufs=4)

        # X1 lives on the right stack from the very start (residual source,
        # then updated in place by proj, read by LN2 and ff2).
        p_x1 = tc.alloc_tile_pool(name="p_x1", bufs=1, side="right")
        X1 = p_x1.tile([P, RT, D], F32)
        nc.sync.dma_start(out=X1, in_=X_r)

        def layer_norm(x_tile, out_bf):
            """x_tile [128, D] f32 -> out_bf [128, D] bf16 normalized,
            WITHOUT gamma/beta (applied later per-partition in T space)."""
            stats = small.tile([P, 2, 6], F32, tag="ln_st", name="st")
            for a in range(2):
                nc.vector.bn_stats(stats[:, a], x_tile[:, a * 512:(a + 1) * 512])
            mv = small.tile([P, 2], F32, tag="ln_mv", name="mv")
            nc.vector.bn_aggr(mv, stats)
            rs = small.tile([P, 1], F32, tag="ln_rs", name="rs")
            nc.vector.tensor_scalar_add(rs, mv[:, 1:2], EPS)
            nc.scalar.activation(rs, rs, AF.Sqrt)
            nc.vector.reciprocal(rs, rs)
            nc.vector.tensor_scalar(out_bf, x_tile, mv[:, 0:1], rs,
                                    op0=ALU.subtract, op1=ALU.mult)

        def transpose_ln(src_bf, dstT, rt, g_pm, b_pm):
            """src_bf [128, D] bf16 (rows rt) -> dstT[dc][:, rt*P:+P] with
            per-partition gamma/beta applied during PSUM->SBUF copyback."""
            for dc in range(DC):
                pt = psum.tile([P, P], BF16, tag="tr", name="pt")
                nc.tensor.transpose(pt, src_bf[:, dc * P:(dc + 1) * P], ident)
                nc.scalar.activation(
                    dstT[dc][:, rt * P:(rt + 1) * P], pt, AF.Identity,
                    bias=b_pm[:, dc:dc + 1], scale=g_pm[:, dc:dc + 1])

        # ================= Phase 1: LN1 + transpose =================
        p_xnT = tc.alloc_tile_pool(name="p_xnT", bufs=1)
        XnT = [p_xnT.tile([P, M], BF16, name=f"XnT{i}") for i in range(DC)]

        for rt in range(RT):
            xn_bf = small.tile([P, D], BF16, tag="xn_bf", name="xnbf", bufs=2)
            layer_norm(X1[:, rt], xn_bf)
            transpose_ln(xn_bf, XnT, rt, gb1_pm, bb1_pm)

        # ================= Phase 2: q,k then v =================
        p_qkv = tc.alloc_tile_pool(name="p_qkv", bufs=1, side="right")
        qT = [p_qkv.tile([P, M], BF16, name=f"qT{i}") for i in range(DC)]
        kT = [p_qkv.tile([P, M], BF16, name=f"kT{i}") for i in range(DC)]
        vsb = [p_qkv.tile([P, D], BF16, name=f"vsb{i}") for i in range(RT)]

        p_wqk = tc.alloc_tile_pool(name="p_wqk", bufs=2)
        p_wv = tc.alloc_tile_pool(name="p_wv", bufs=2)
        for g in range(4):  # feature groups of 512: g0,g1 -> q; g2,g3 -> k
            wg = p_wqk.tile([P, DC, 512], BF16, tag="wqk", name="wg")
            nc.sync.dma_start(out=wg, in_=Wqkv_r[:, :, g * 512:(g + 1) * 512])
            for fo in range(4):
                fc = g * 4 + fo
                dst = qT[fc] if fc < 8 else kT[fc - 8]
                for rh in range(2):
                    pm = psum.tile([P, 512], F32, tag="mm", name="pm")
                    for kc in range(DC):
                        nc.tensor.matmul(
                            pm, lhsT=wg[:, kc, fo * P:(fo + 1) * P],
                            rhs=XnT[kc][:, rh * 512:(rh + 1) * 512],
                            start=(kc == 0), stop=(kc == DC - 1))
                    nc.vector.tensor_scalar_add(
                        dst[:, rh * 512:(rh + 1) * 512], pm,
                        bqk_pm[:, fc:fc + 1])
        for vh in range(2):
            wv = p_wv.tile([P, DC, 512], BF16, tag="wv", name="wv")
            nc.sync.dma_start(
                out=wv, in_=Wqkv_r[:, :, 2 * D + vh * 512:2 * D + (vh + 1) * 512])
            for rt in range(RT):
                pm = psum.tile([P, 512], F32, tag="mm", name="pm")
                for kc in range(DC):
                    nc.tensor.matmul(
                        pm, lhsT=XnT[kc][:, rt * P:(rt + 1) * P],
                        rhs=wv[:, kc],
                        start=(kc == 0), stop=(kc == DC - 1))
                nc.vector.tensor_add(
                    vsb[rt][:, vh * 512:(vh + 1) * 512], pm,
                    bv_bc[:, vh * 512:(vh + 1) * 512])
        p_wv.release()
        p_wqk.release()
        p_xnT.release()

        # ========== Phase 3/4: attention + proj interleaved per batch ==========
        p_ctxT = tc.alloc_tile_pool(name="p_ctxT", bufs=1)
        ctxT = [p_ctxT.tile([P, M], BF16, name=f"ctxT{i}") for i in range(DC)]
        p_A = tc.alloc_tile_pool(name="p_A", bufs=1)
        A_acc = [p_A.tile([P, C], F32, name=f"Aacc{i}") for i in range(2 * NB)]
        p_wproj = tc.alloc_tile_pool(name="p_wproj", bufs=1)
        Wproj_sb = p_wproj.tile([P, DC, D], BF16)
        nc.sync.dma_start(out=Wproj_sb, in_=Wproj_r)
        p_attn = tc.alloc_tile_pool(name="p_attn", bufs=4)

        for b in range(NB):
            for h in range(H):
                th, po = (h * DH) // P, (h * DH) % P
                q_l = qT[th][po:po + DH, b * C:(b + 1) * C]
                k_l = kT[th][po:po + DH, b * C:(b + 1) * C]
                attn_bf = p_attn.tile([P, 2, C], BF16, tag="attn_bf", name="abf")
                for cc in range(2):
                    ps = psum.tile([P, C], F32, tag="scores", name="ps", bufs=3)
                    nc.tensor.matmul(ps, lhsT=q_l[:, cc * P:(cc + 1) * P],
                                     rhs=k_l, start=True, stop=True)
                    es = p_attn.tile([P, C], F32, tag="expS", name="es")
                    den = small.tile([P, 1], F32, tag="den", name="den")
                    nc.scalar.activation(es, ps, AF.Exp, scale=SCALE,
                                         accum_out=den)
                    rden = small.tile([P, 1], F32, tag="rden", name="rden")
                    nc.vector.reciprocal(rden, den)
                    nc.vector.tensor_scalar_mul(attn_bf[:, cc], es, rden)
                    # A accumulation (f32): A += es * rden
                    a_t = A_acc[b * 2 + cc]
                    if h == 0:
                        nc.vector.tensor_scalar_mul(a_t, es, rden)
                    else:
                        nc.vector.scalar_tensor_tensor(
                            out=a_t, in0=es, scalar=rden, in1=a_t,
                            op0=ALU.mult, op1=ALU.add)
                # transpose attn -> attnT [k on partitions, c free]
                attnT = p_attn.tile([P, 2, C], BF16, tag="attnT", name="atT")
                for kc2 in range(2):
                    for cc in range(2):
                        pt = psum.tile([P, P], BF16, tag="tr", name="pt")
                        nc.tensor.transpose(
                            pt, attn_bf[:, cc, kc2 * P:(kc2 + 1) * P], ident)
                        nc.any.tensor_copy(
                            out=attnT[:, kc2, cc * P:(cc + 1) * P], in_=pt)
                # ctx^T chunk [DH, C]
                pc = psum.tile([DH, C], F32, tag="ctx", name="pc", bufs=1)
                for kc2 in range(2):
                    nc.tensor.matmul(
                        pc, lhsT=vsb[b * 2 + kc2][:, h * DH:(h + 1) * DH],
                        rhs=attnT[:, kc2],
                        start=(kc2 == 0), stop=(kc2 == 1))
                nc.any.tensor_copy(
                    out=ctxT[th][po:po + DH, b * C:(b + 1) * C], in_=pc)

            # A outputs for batch b
            for cc in range(2):
                i = b * 2 + cc
                nc.vector.tensor_scalar_mul(A_acc[i], A_acc[i], 1.0 / H)
                nc.sync.dma_start(out=A_out[i * P:(i + 1) * P, :], in_=A_acc[i])

            # proj for batch b's two row tiles (fills PE while next batch's
            # softmax chains run)
            for rt in (2 * b, 2 * b + 1):
                for nh2 in range(2):
                    sl = slice(nh2 * 512, (nh2 + 1) * 512)
                    pm = psum.tile([P, 512], F32, tag="mm", name="pm")
                    for dc in range(DC):
                        nc.tensor.matmul(
                            pm, lhsT=ctxT[dc][:, rt * P:(rt + 1) * P],
                            rhs=Wproj_sb[:, dc, sl],
                            start=(dc == 0), stop=(dc == DC - 1))
                    nc.vector.tensor_add(X1[:, rt, sl], X1[:, rt, sl], pm)
                    nc.vector.tensor_add(X1[:, rt, sl], X1[:, rt, sl],
                                         bproj_bc[:, sl])
        p_attn.release()
        p_wproj.release()
        p_A.release()
        p_ctxT.release()
        p_qkv.release()

        # ================= Phase 5: LN2 + transpose =================
        # hT goes on the right stack now (before wff2) so both can prefetch.
        p_hT = tc.alloc_tile_pool(name="p_hT", bufs=1, side="right")
        hT = [p_hT.tile([P, M], BF16, name=f"hT{i}") for i in range(FC)]
        p_wff2 = tc.alloc_tile_pool(name="p_wff2", bufs=1, side="right")
        Wff2_sb = p_wff2.tile([P, FC, D], BF16)
        for kcg in range(4):
            nc.sync.dma_start(out=Wff2_sb[:, kcg * 8:(kcg + 1) * 8, :],
                              in_=Wff2_r[:, kcg * 8:(kcg + 1) * 8, :])

        p_x2nT = tc.alloc_tile_pool(name="p_x2nT", bufs=1)
        X2nT = [p_x2nT.tile([P, M], BF16, name=f"X2nT{i}") for i in range(DC)]
        p_wff1 = tc.alloc_tile_pool(name="p_wff1", bufs=2)
        for rt in range(RT):
            x2n_bf = small.tile([P, D], BF16, tag="xn_bf", name="x2nbf", bufs=2)
            layer_norm(X1[:, rt], x2n_bf)
            transpose_ln(x2n_bf, X2nT, rt, gb2_pm, bb2_pm)

        # ================= Phase 6: ff1 + gelu -> hT =================
        for g in range(16):  # dff groups of 256
            wg = p_wff1.tile([P, DC, 256], BF16, tag="wff1", name="wf1")
            nc.sync.dma_start(out=wg, in_=Wff1_r[:, :, g * 256:(g + 1) * 256])
            for fo in range(2):
                fc = g * 2 + fo
                for rh in range(2):
                    pm = psum.tile([P, 512], F32, tag="mm", name="pm")
                    for kc in range(DC):
                        nc.tensor.matmul(
                            pm, lhsT=wg[:, kc, fo * P:(fo + 1) * P],
                            rhs=X2nT[kc][:, rh * 512:(rh + 1) * 512],
                            start=(kc == 0), stop=(kc == DC - 1))
                    nc.scalar.activation(
                        hT[fc][:, rh * 512:(rh + 1) * 512], pm, AF.Gelu,
                        bias=bff1_pm[:, fc:fc + 1])
        p_wff1.release()
        p_x2nT.release()

        # ================= Phase 7: ff2 + residual -> X_out =================
        p_out = tc.alloc_tile_pool(name="p_out", bufs=3)
        for rt in range(RT):
            out_t = p_out.tile([P, D], F32, tag="out_t", name="outt")
            for nh2 in range(2):
                sl = slice(nh2 * 512, (nh2 + 1) * 512)
                pm = psum.tile([P, 512], F32, tag="mm", name="pm")
                for fc in range(FC):
                    nc.tensor.matmul(
                        pm, lhsT=hT[fc][:, rt * P:(rt + 1) * P],
                        rhs=Wff2_sb[:, fc, sl],
                        start=(fc == 0), stop=(fc == FC - 1))
                nc.vector.tensor_add(out_t[:, sl], X1[:, rt, sl], pm)
                nc.vector.tensor_add(out_t[:, sl], out_t[:, sl],
                                     bff2_bc[:, sl])
            nc.sync.dma_start(out=X_out[rt * P:(rt + 1) * P, :], in_=out_t)
        p_out.release()
        p_wff1_dummy = None  # placeholder, wff1 released above
        p_x2nT_dummy = None
        p_wff2.release()
        p_hT.release()
        p_x1.release()
        small.release()
        p_ident.release()
        psum.release()

    return nc


# ======================== host-side wrapper ========================
import sys as _sys
for _p in ("/opt/trn_rl_repo", "/root/.axon_site/_ro/trn_rl_repo"):
    if _p not in _sys.path:
        _sys.path.insert(0, _p)

import numpy as _np
import ml_dtypes as _mld

_N_CORES = 8
_B_FULL = 32

_COMPILED = {}


def _get_compiled(repeat=1):
    if repeat not in _COMPILED:
        import concourse.bacc as _bacc
        _nc = _bacc.Bacc("TRN2", target_bir_lowering=False, debug=False)
        build_kernel(_nc, repeat=repeat)
        _nc.compile()
        _COMPILED[repeat] = _nc
    return _COMPILED[repeat]


def _bf16(a):
    return _np.ascontiguousarray(a).astype(_mld.bfloat16)


def _make_in_maps(inputs):
    shared = {
        "W_qkv": _bf16(inputs["W_qkv"]), "b_qkv": _np.asarray(inputs["b_qkv"], _np.float32),
        "W_proj": _bf16(inputs["W_proj"]), "b_proj": _np.asarray(inputs["b_proj"], _np.float32),
        "g1": _np.asarray(inputs["g1"], _np.float32),
        "beta1": _np.asarray(inputs["beta1"], _np.float32),
        "g2": _np.asarray(inputs["g2"], _np.float32),
        "beta2": _np.asarray(inputs["beta2"], _np.float32),
        "W_ff1": _bf16(inputs["W_ff1"]), "b_ff1": _np.asarray(inputs["b_ff1"], _np.float32),
        "W_ff2": _bf16(inputs["W_ff2"]), "b_ff2": _np.asarray(inputs["b_ff2"], _np.float32),
    }
    X_full = _np.asarray(inputs["X"], _np.float32)
    nb = _B_FULL // _N_CORES
    in_maps = []
    for c in range(_N_CORES):
        xs = X_full[c * nb:(c + 1) * nb].reshape(nb * C, D)
        in_maps.append({"X": _np.ascontiguousarray(xs), **shared})
    return in_maps


def kernel(**inputs):
    """Full-input entry point: shards batch over 8 NeuronCores (data
    parallel), runs the Bass kernel, gathers full outputs.

    Returns (X_out [32,256,1024] f32, A [32,256,256] f32) matching the
    reference's return structure."""
    from concourse import bass_utils

    nc = _get_compiled(repeat=1)
    in_maps = _make_in_maps(inputs)
    res = bass_utils.run_bass_kernel_spmd(
        nc, in_maps, core_ids=list(range(_N_CORES)))
    nb = _B_FULL // _N_CORES
    X_out = _np.concatenate(
        [res.results[c]["X_out"].reshape(nb, C, D) for c in range(_N_CORES)])
    A = _np.concatenate(
        [res.results[c]["A_out"].reshape(nb, C, C) for c in range(_N_CORES)])
    return X_out, A
